# revision 1
# baseline (speedup 1.0000x reference)
"""Trainium2 Bass kernel for nn_LungCancerGRU (GRU H=64, T=15, B=262144 -> logits [B,2]).

Data parallel over 8 NeuronCores (batch sharded, 32768 rows/core).

Per-core layout is "transposed": gate units on SBUF partitions, batch on the
free dimension.  Batch runs in pair-tiles of 1024 rows = two groups (A, B) of
N=512; group A occupies partitions 0..63, group B 64..127 of every [128, 512]
tile, so all engines run at full 128-partition width.

Per timestep t (per pair-tile):
  psum_rz[:, :512] = BD(W_hr^T) @ h + x2_r @ [x_t^A; x_t^B]   (r preact)
  psum_rz[:, 512:] = BD(W_hz^T) @ h + x2_z @ ...              (z preact)
  psum_hgn         = BD(W_hn^T) @ h + b_hh_n (K=1 ones row)   (h-part of n gate)
  psum_n           = x2_n @ [x_t^A; x_t^B]                    (x-part of n gate)
  z   = sigmoid(psum_rz[:, 512:] + bias_z)   ACT, per-partition bias
  r   = sigmoid(psum_rz[:, :512] + bias_r)
  hgn = copy(psum_hgn)                       DVE psum->bf16
  m1  = hgn * r                              DVE bf16 2x
  psum_n += I128 @ m1                        identity-matmul accumulate (PE)
  n   = tanh(psum_n + bias_n)                ACT
  u = h - n; v = z*u; h' = n + v             DVE bf16 2x

BD(W) = blockdiag(W, W) [128,128] serves both groups per matmul stream.  All
biases fold into PE matmuls / ACT per-partition bias vectors.  The rank-1 x
contribution rides a K=2 accumulating matmul from an x-transpose tile
xt [2, 512*15] (layout [group, n*15+t]) loaded by one fully-contiguous DMA.

Hardware constraint honored throughout: each instruction can carry at most
ONE semaphore wait (matmuls: two, split onto LDWEIGHTS).  InstNoOp
"observer" pseudo-instructions (APs used only for dependency wiring, dropped
at lowering) advance each engine's vector clock once per step so that no
real instruction ever needs a second wait.

FC head: logitsT [2,512] per group via PE (stationary W_fc^T), bias added in
the PSUM->SBUF tensor_scalar copy (per-partition bias), DMA'd out through a
transposing access pattern.
"""

import sys

import numpy as np

sys.path.insert(0, "/opt/trn_rl_repo")

B, T, H = 262144, 15, 64
NCORES = 8
BC = B // NCORES          # 32768 rows per core
N = 512                   # batch columns per group
PAIR = 2 * N              # 1024 rows per pair-tile
NPAIR = BC // PAIR        # 32 pair-tiles per core
IL = 2                    # pair-tiles processed in lockstep
XW = T * N                # xt tile free width (7680)

# f32 const tile column map (cf [128, CF_W])
CF_X2 = 0                 # cols 0:384, partitions 0:2 - x2 lhsT per gate
CF_BHHN = 384
CF_BR = 385
CF_BZ = 386
CF_BN = 387
CF_WFC = 388              # cols 388:390
CF_BFC = 390              # col 390, partitions 0:2
CF_ONES = 392             # cols 392:904, partition 0 - ones rhs row
CF_BH2 = 904              # cols 904:1032, partition 0 - [b_hh_n | b_hh_n] lhsT row
CF_W = 1032

_cache = {}


def _build(dt_h_name, reps=1):
    from contextlib import ExitStack

    import concourse.bacc as bacc
    import concourse.mybir as mybir
    from concourse.tile import TileContext

    f32 = mybir.dt.float32
    dt_h = getattr(mybir.dt, dt_h_name)
    Act = mybir.ActivationFunctionType
    Alu = mybir.AluOpType

    nc = bacc.Bacc(None)

    x_in = nc.dram_tensor("x", [BC, T], f32, kind="ExternalInput")
    out_d = nc.dram_tensor("out", [BC, 2], f32, kind="ExternalOutput")
    cbf_in = nc.dram_tensor("cbf", [128, 512], dt_h, kind="ExternalInput")
    cf_in = nc.dram_tensor("cf", [128, CF_W], f32, kind="ExternalInput")

    with TileContext(nc) as tc, ExitStack() as es:
        # ---- constants: one DMA per dtype ----
        cpool = es.enter_context(tc.tile_pool(name="const", bufs=1))
        cbf = cpool.tile([128, 512], dt_h)
        nc.sync.dma_start(cbf[:], cbf_in[:])
        cf = cpool.tile([128, CF_W], f32)
        nc.sync.dma_start(cf[:], cf_in[:])

        bd_g = [cbf[:, 128 * g:128 * (g + 1)] for g in range(3)]
        i128 = cbf[:, 384:512]
        x2_g = [cf[0:2, 128 * g:128 * (g + 1)] for g in range(3)]
        bias_r = cf[:, CF_BR:CF_BR + 1]
        bias_z = cf[:, CF_BZ:CF_BZ + 1]
        bias_n = cf[:, CF_BN:CF_BN + 1]
        wfc = cf[:, CF_WFC:CF_WFC + 2]
        bfc = cf[0:2, CF_BFC:CF_BFC + 1]
        ones_row = cf[0:1, CF_ONES:CF_ONES + N]
        bh2_row = cf[0:1, CF_BH2:CF_BH2 + 128]

        # ---- pools ----
        xt_pool = es.enter_context(tc.tile_pool(name="xt", bufs=3))
        hp = es.enter_context(tc.tile_pool(name="h", bufs=2 * IL))
        hf = es.enter_context(tc.tile_pool(name="hf", bufs=IL))
        rzp = es.enter_context(tc.tile_pool(name="rz", bufs=2 * IL))
        hgp = es.enter_context(tc.tile_pool(name="hg", bufs=2 * IL))
        m1p = es.enter_context(tc.tile_pool(name="m1", bufs=2 * IL))
        np_ = es.enter_context(tc.tile_pool(name="nt", bufs=2 * IL))
        up = es.enter_context(tc.tile_pool(name="u", bufs=2 * IL))
        wp = es.enter_context(tc.tile_pool(name="w", bufs=2 * IL))
        zhp = es.enter_context(tc.tile_pool(name="zh", bufs=2 * IL))
        stp = es.enter_context(tc.tile_pool(name="stage", bufs=2 * IL))
        prz = es.enter_context(tc.tile_pool(name="prz", bufs=2, space="PSUM"))
        pn = es.enter_context(tc.tile_pool(name="pn", bufs=2, space="PSUM"))
        phgn = es.enter_context(tc.tile_pool(name="phgn", bufs=2, space="PSUM"))
        plog = pn  # FC logits rotate through the pn slots (shared tag)

        def mm(out, lhsT, rhs, start, stop):
            nc.tensor.matmul(out, lhsT, rhs, start=start, stop=stop,
                             skip_group_check=True)

        # ---- engine warm-ups: fold the const-DMA sems into each engine's
        # clock once so steady-state instructions never re-wait on them.
        pwarm = plog.tile([2, 2], f32, tag="pn")
        mm(pwarm[:], cf[0:2, 0:2], cf[0:2, 0:2], True, True)
        pwarm2 = plog.tile([2, 2], f32, tag="pn")
        mm(pwarm2[:], cbf[0:2, 0:2], cbf[0:2, 0:2], True, True)
        wt = cpool.tile([2, 8], f32)
        nc.vector.tensor_copy(wt[0:1, 0:1], cf[0:1, 0:1])
        nc.vector.tensor_copy(wt[0:1, 1:2], cbf[0:1, 0:1])
        nc.scalar.copy(wt[0:1, 2:3], cf[0:1, 0:1])
        nc.scalar.copy(wt[0:1, 3:4], cbf[0:1, 0:1])

        def stage_a(pr, t):
            """PE matmuls + sigmoids + psum->sbuf copy of hgn."""
            h = pr["h"]
            xcols = pr["xtv"][:, t, :]
            p_rz = prz.tile([128, 2 * N], f32, tag="prz")
            p_n = pn.tile([128, N], f32, tag="pn")
            p_h = phgn.tile([128, N], f32, tag="phgn")
            if h is not None:
                mm(p_rz[:, 0:N], bd_g[0], h[:], True, False)
                mm(p_rz[:, 0:N], x2_g[0], xcols, False, True)
                mm(p_rz[:, N:2 * N], bd_g[1], h[:], True, False)
                mm(p_rz[:, N:2 * N], x2_g[1], xcols, False, True)
                mm(p_h[:], bd_g[2], h[:], True, False)
                mm(p_h[:], bh2_row, ones_row, False, True)
            else:
                mm(p_rz[:, 0:N], x2_g[0], xcols, True, True)
                mm(p_rz[:, N:2 * N], x2_g[1], xcols, True, True)
                mm(p_h[:], bh2_row, ones_row, True, True)
            mm(p_n[:], x2_g[2], xcols, True, False)

            rz = rzp.tile([128, 2 * N + 8], dt_h, tag="rz")
            if pr["absorb"] is not None:
                # tiny first-toucher: carries this tile's WAR and advances
                # ACT's DVE clock past the newest h tick -> all other ACT ops
                # this step keep a single PE wait.
                nc.scalar.copy(rz[0:1, 2 * N:2 * N + 1], pr["absorb"][0:1, 0:1])
            # r first (the critical path runs through it), then z
            nc.scalar.activation(rz[:, 0:N], p_rz[:, 0:N], Act.Sigmoid, bias=bias_r)
            nc.scalar.activation(rz[:, N:2 * N], p_rz[:, N:2 * N], Act.Sigmoid,
                                 bias=bias_z)
            hgn_sb = hgp.tile([128, N + 8], dt_h, tag="hg")
            nc.vector.tensor_copy(hgn_sb[:, 0:N], p_h[:])
            # advance DVE's ACT clock past sigmoid_r before m1 runs
            nc.vector.tensor_copy(hgn_sb[0:1, N:N + 1], rz[0:1, 0:1])
            pr["p_n"], pr["rz"], pr["hgn"] = p_n, rz, hgn_sb

        def stage_b(pr, t):
            """m1 multiply, identity accumulate, tanh; off-chain w and z*h."""
            p_n, rz, hgn_sb = pr["p_n"], pr["rz"], pr["hgn"]
            m1 = m1p.tile([128, N], dt_h, tag="m1")
            nc.vector.tensor_tensor(m1[:], hgn_sb[:, 0:N], rz[:, 0:N], Alu.mult)
            mm(p_n[:], i128, m1[:], False, True)
            # off the critical path: w = 1 - z, zh = z * h
            w = wp.tile([128, N], dt_h, tag="w")
            nc.vector.tensor_scalar(w[:], rz[:, N:2 * N], -1.0, 1.0,
                                    Alu.mult, Alu.add)
            if pr["h"] is not None:
                zh = zhp.tile([128, N], dt_h, tag="zh")
                nc.vector.tensor_tensor(zh[:], rz[:, N:2 * N], pr["h"][:], Alu.mult)
                pr["zh"] = zh
            else:
                pr["zh"] = None
            pr["w"] = w
            n_t = np_.tile([128, N], dt_h, tag="nt")
            nc.scalar.activation(n_t[:], p_n[:], Act.Tanh, bias=bias_n)
            pr["n_t"] = n_t

        def stage_c(pr, t):
            """h' = n*(1-z) + z*h   (zh precomputed off-chain)."""
            n_t, w, zh = pr["n_t"], pr["w"], pr["zh"]
            last = t == T - 1
            if last:
                h_new = hf.tile([128, N], f32, tag="hf")
            else:
                h_new = hp.tile([128, N], dt_h, tag="h")
            if zh is not None:
                nw = up.tile([128, N], dt_h, tag="nw")
                nc.vector.tensor_tensor(nw[:], n_t[:], w[:], Alu.mult)
                nc.vector.tensor_tensor(h_new[:], nw[:], zh[:], Alu.add)
            else:
                nc.vector.tensor_tensor(h_new[:], n_t[:], w[:], Alu.mult)
            pr["h"] = h_new

        def fc_out(pr, blk):
            h = pr["h"]
            base = pr["base"]
            st = pr["st"]
            for g in range(2):
                p_l = plog.tile([2, N], f32, tag="pn")
                mm(p_l[:], wfc[64 * g:64 * (g + 1), :], h[64 * g:64 * (g + 1), :],
                   True, True)
                stg = st[0:2, g * N:(g + 1) * N]
                nc.vector.tensor_scalar(stg, p_l[:], bfc, None, Alu.add)
            # one DMA for the whole pair: rows base..base+1024 in DRAM match
            # the st column order exactly
            rows = out_d[base:base + PAIR, :]
            nc.sync.dma_start(rows.rearrange("n two -> two n"), st[0:2, :])

        prev_hf = None
        for blk in range(reps * (NPAIR // IL)):
            blk = blk % (NPAIR // IL)
            pairs = []
            for j in range(IL):
                p = blk * IL + j
                base = p * PAIR
                xt = xt_pool.tile([2, XW], f32)
                # one DMA, contiguous innermost on both sides:
                # xt[g, n*T + t] <- x[base + g*N + n, t]
                src = x_in[base:base + PAIR, :]
                nc.sync.dma_start(
                    xt[:].rearrange("g (n t) -> g n t", t=T),
                    src.rearrange("(g n) t -> g n t", g=2))
                st = stp.tile([2, 2 * N], f32, tag="st")
                # tiny first-toucher carries the WAR on the old out-DMA, so
                # the staging writes later keep a single PE wait
                nc.vector.tensor_copy(st[0:1, 0:1], cf[0:1, 0:1])
                pairs.append({"xtv": xt[:].rearrange("g (n t) -> g t n", t=T),
                              "xt": xt, "base": base, "h": None, "st": st,
                              "absorb": None})
            for t in range(T):
                # newest DVE-written tile: h of the last pair (or previous
                # superblock's final h at t=0)
                newest = pairs[-1]["h"] if t > 0 else prev_hf
                pairs[0]["absorb"] = newest
                pairs[1]["absorb"] = None
                for pr in pairs:
                    stage_a(pr, t)
                for pr in pairs:
                    stage_b(pr, t)
                for pr in pairs:
                    stage_c(pr, t)
            for pr in pairs:
                fc_out(pr, blk)
            prev_hf = pairs[-1]["h"]

    nc.compile()
    return nc


def _host_constants(W_ih, W_hh, b_ih, b_hh, W_fc, b_fc, dt_h_np):
    f32 = np.float32
    cbf = np.zeros((128, 512), f32)
    cf = np.zeros((128, CF_W), f32)
    w_in = W_ih[:, 0].astype(f32)
    for g in range(3):
        W = W_hh[64 * g:64 * (g + 1)].astype(f32)          # [64, 64]
        cbf[0:64, 128 * g:128 * g + 64] = W.T
        cbf[64:128, 128 * g + 64:128 * g + 128] = W.T
        wg = w_in[64 * g:64 * (g + 1)]
        cf[0, 128 * g:128 * g + 64] = wg
        cf[1, 128 * g + 64:128 * g + 128] = wg
    cbf[:, 384:512] = np.eye(128, dtype=f32)
    cf[:, CF_BHHN] = np.concatenate([b_hh[128:192]] * 2)
    cf[:, CF_BR] = np.concatenate([(b_ih[0:64] + b_hh[0:64])] * 2)
    cf[:, CF_BZ] = np.concatenate([(b_ih[64:128] + b_hh[64:128])] * 2)
    cf[:, CF_BN] = np.concatenate([b_ih[128:192]] * 2)
    cf[0:64, CF_WFC:CF_WFC + 2] = W_fc.T
    cf[64:128, CF_WFC:CF_WFC + 2] = W_fc.T
    cf[0:2, CF_BFC] = b_fc
    cf[0, CF_ONES:CF_ONES + N] = 1.0
    cf[0, CF_BH2:CF_BH2 + 128] = np.concatenate([b_hh[128:192]] * 2)
    return {"cbf": cbf.astype(dt_h_np), "cf": cf}


def kernel(x, W_ih, W_hh, b_ih, b_hh, W_fc, b_fc, _trace=False, _trace_kwargs=None):
    import ml_dtypes

    from concourse.bass_utils import run_bass_kernel_spmd

    dt_h_name = "bfloat16"
    if dt_h_name not in _cache:
        _cache[dt_h_name] = _build(dt_h_name)
    nc = _cache[dt_h_name]

    consts = _host_constants(W_ih, W_hh, b_ih, b_hh, W_fc, b_fc,
                             ml_dtypes.bfloat16)
    x = np.ascontiguousarray(np.asarray(x, np.float32))
    in_maps = []
    for c in range(NCORES):
        m = {"x": x[c * BC:(c + 1) * BC]}
        m.update(consts)
        in_maps.append(m)
    kw = {}
    if _trace:
        kw["trace"] = True
        if _trace_kwargs:
            kw.update(_trace_kwargs)
    res = run_bass_kernel_spmd(nc, in_maps, list(range(NCORES)), **kw)
    out = np.concatenate([res.results[c]["out"] for c in range(NCORES)], axis=0)
    if _trace:
        return out, res
    return out


if __name__ == "__main__":
    rng = np.random.default_rng(0)
    s = 1.0 / np.sqrt(H)
    inputs = {
        "x": rng.standard_normal((B, T), dtype=np.float32),
        "W_ih": rng.uniform(-s, s, (3 * H, 1)).astype(np.float32),
        "W_hh": rng.uniform(-s, s, (3 * H, H)).astype(np.float32),
        "b_ih": rng.uniform(-s, s, (3 * H,)).astype(np.float32),
        "b_hh": rng.uniform(-s, s, (3 * H,)).astype(np.float32),
        "W_fc": rng.uniform(-s, s, (2, H)).astype(np.float32),
        "b_fc": rng.uniform(-s, s, (2,)).astype(np.float32),
    }
    out = kernel(**inputs)
    print(out.shape, out.dtype, out[:4])



# revision 5
# speedup vs baseline: 1.9132x; 1.9132x over previous
"""Trainium2 Bass kernel for nn_LungCancerGRU (GRU H=64, T=15, B=262144 -> logits [B,2]).

Data parallel over 8 NeuronCores (batch sharded, 32768 rows/core).

Per-core layout: gate units on SBUF partitions, batch on the free dimension.
Batch runs in pair-tiles of 1024 rows = two groups (A, B) of N=512; group A
occupies partitions 0..63, group B 64..127 of every [128, 512] tile, so all
engines run at full 128-partition width.  All matmul operands are bf16
(fp32 matmuls run at 1/4 PE rate - avoid entirely).

Per timestep t (per pair-tile), with z sign-flipped host-side so the single
sigmoid yields w = 1-z directly:
  psum_rz[:, :512] = BD(W_hr^T) @ h + x3_r @ [x^A; x^B; 1]    (r preact+bias)
  psum_rz[:, 512:] = BD(-W_hz^T) @ h + x3_z @ ...             (-z preact-bias)
  psum_nh          = BD(W_hn^T) @ h                           (h-part of n)
  psum_n           = x3_n @ [x^A; x^B; 1]                     (x w_n + b_ih_n)
  rz  = sigmoid(psum_rz)    ONE ACT op [128,1024] -> [r | w]
  m1  = (psum_nh + b_hhn) * r          fused scalar_tensor_tensor on DVE
  psum_n += I128 @ m1                  identity-matmul accumulate (PE)
  n   = tanh(psum_n)                   ACT
  u = n - h (GpSimd); v = w*u (GpSimd); h' = h + v (DVE)

Biases ride a constant-1.0 third row of the x-transpose tile xt [3, 512*15]
(layout [group, n*15+t]; row 2 all ones) through the K=3 x-part matmuls.
The xt tiles are 3 persistent slots; x is cast f32->bf16 in the SWDGE DMA.

FC head: logitsT [2,512] per group via PE (stationary W_fc^T bf16), bias
added in the PSUM->SBUF tensor_scalar copy.  Output is stored TRANSPOSED in
DRAM as [2, BC] (contiguous row DMAs; the strided [BC,2] layout costs 22us
per DMA) and transposed back on the host.
"""

import sys

import numpy as np

sys.path.insert(0, "/opt/trn_rl_repo")

B, T, H = 262144, 15, 64
NCORES = 8
BC = B // NCORES          # 32768 rows per core
N = 512                   # batch columns per group
PAIR = 2 * N              # 1024 rows per pair-tile
NPAIR = BC // PAIR        # 32 pair-tiles per core
IL = 2                    # pair-tiles processed in lockstep
NSB = NPAIR // IL         # superblocks
XW = T * N                # xt tile free width (7680)

_cache = {}


def _build(dt_h_name):
    from contextlib import ExitStack

    import concourse.bacc as bacc
    import concourse.mybir as mybir
    from concourse.tile import TileContext

    f32 = mybir.dt.float32
    dt_h = getattr(mybir.dt, dt_h_name)
    Act = mybir.ActivationFunctionType
    Alu = mybir.AluOpType

    nc = bacc.Bacc(None)

    x_in = nc.dram_tensor("x", [BC, T], f32, kind="ExternalInput")
    out_d = nc.dram_tensor("out", [2, BC], f32, kind="ExternalOutput")
    cb_in = nc.dram_tensor("cb", [128, 1024], dt_h, kind="ExternalInput")
    cf_in = nc.dram_tensor("cf", [128, 8], f32, kind="ExternalInput")

    with TileContext(nc) as tc, ExitStack() as es:
        # ---- constants: one DMA per dtype ----
        cpool = es.enter_context(tc.tile_pool(name="const", bufs=1))
        cb = cpool.tile([128, 1024], dt_h)
        nc.sync.dma_start(cb[:], cb_in[:])
        cf = cpool.tile([128, 8], f32)
        nc.sync.dma_start(cf[:], cf_in[:])

        bd = [cb[:, 128 * g:128 * (g + 1)] for g in range(3)]  # r, -z, n
        i128 = cb[:, 384:512]
        wfc = cb[:, 512:514]
        x3 = [cb[0:3, 640 + 128 * g:768 + 128 * g] for g in range(3)]
        bhn = cf[:, 0:1]
        bfc = cf[0:2, 1:2]

        # ---- pools ----
        xt_pool = es.enter_context(tc.tile_pool(name="xt", bufs=3))
        hp = es.enter_context(tc.tile_pool(name="h", bufs=2 * IL))
        hfp = es.enter_context(tc.tile_pool(name="hf", bufs=IL))
        rzp = es.enter_context(tc.tile_pool(name="rz", bufs=2 * IL))
        m1p = es.enter_context(tc.tile_pool(name="m1", bufs=2 * IL))
        nsp = es.enter_context(tc.tile_pool(name="ns", bufs=2 * IL))
        upl = es.enter_context(tc.tile_pool(name="u", bufs=2 * IL))
        vpl = es.enter_context(tc.tile_pool(name="v", bufs=2 * IL))
        stp = es.enter_context(tc.tile_pool(name="st", bufs=2))
        przp = es.enter_context(tc.tile_pool(name="prz", bufs=2, space="PSUM"))
        pnhp = es.enter_context(tc.tile_pool(name="pnh", bufs=2, space="PSUM"))
        pnp = es.enter_context(tc.tile_pool(name="pn", bufs=2, space="PSUM"))

        def mm(out, lhsT, rhs, start, stop):
            nc.tensor.matmul(out, lhsT, rhs, start=start, stop=stop,
                             skip_group_check=True)

        # ---- persistent xt tiles; row 2 = 1.0 (bias lane), set once ----
        xts = []
        for i in range(3):
            xt = xt_pool.tile([3, XW], dt_h, tag="xt")
            # rows 0:2 are overwritten by every x DMA; row 2 stays 1.0
            # (engine ops must start at a partition multiple of 32)
            nc.gpsimd.memset(xt[0:3, :], 1.0)
            xts.append(xt)

        # ---- engine warm-ups: fold the const-DMA sems into each engine's
        # clock once so steady-state instructions never re-wait on them.
        pwarm = pnp.tile([2, 2], f32, tag="pn")
        mm(pwarm[:], cb[0:2, 0:2], cb[0:2, 0:2], True, True)
        wt = cpool.tile([2, 8], f32)
        nc.vector.tensor_copy(wt[0:1, 0:1], cf[0:1, 0:1])
        nc.vector.tensor_copy(wt[0:1, 1:2], cb[0:1, 0:1])
        nc.scalar.copy(wt[0:1, 2:3], cf[0:1, 0:1])
        nc.scalar.copy(wt[0:1, 3:4], cb[0:1, 0:1])
        nc.gpsimd.tensor_copy(wt[0:1, 4:5], cb[0:1, 0:1])
        nc.gpsimd.tensor_copy(wt[0:1, 5:6], cf[0:1, 0:1])

        for blk in range(NSB):
            st = stp.tile([2, IL * PAIR], f32, tag="st")
            prs = []
            for j in range(IL):
                p = blk * IL + j
                xt = xts[p % 3]
                # one cast DMA (f32 -> bf16), contiguous on both sides
                src = x_in[p * PAIR:(p + 1) * PAIR, :]
                nc.gpsimd.dma_start(
                    xt[0:2, :].rearrange("g (n t) -> g n t", t=T),
                    src.rearrange("(g n) t -> g n t", g=2))
                prs.append({"xtv": xt[:].rearrange("g (n t) -> g t n", t=T),
                            "h": None, "j": j})
            for t in range(T):
                first = t == 0
                last = t == T - 1
                for pr in prs:
                    pr["prz"] = przp.tile([128, 2 * N], f32, tag="prz", name="prz")
                    pr["pn"] = pnp.tile([128, N], f32, tag="pn", name="pn")
                # ---- PE streams, weight-grouped across the IL pairs ----
                if not first:
                    for pr in prs:
                        mm(pr["prz"][:, 0:N], bd[0], pr["h"][:], True, False)
                    for pr in prs:
                        mm(pr["prz"][:, N:2 * N], bd[1], pr["h"][:], True, False)
                for pr in prs:
                    mm(pr["prz"][:, 0:N], x3[0], pr["xtv"][:, t, :], first, True)
                for pr in prs:
                    mm(pr["prz"][:, N:2 * N], x3[1], pr["xtv"][:, t, :], first, True)
                if not first:
                    for pr in prs:
                        pr["pnh"] = pnhp.tile([128, N], f32, tag="pnh", name="pnh")
                        mm(pr["pnh"][:], bd[2], pr["h"][:], True, True)
                for pr in prs:
                    mm(pr["pn"][:], x3[2], pr["xtv"][:, t, :], True, False)
                # ---- sigmoid -> [r | w],  m1 = (nh + b_hhn) * r ----
                for pr in prs:
                    rz = rzp.tile([128, 2 * N], dt_h, tag="rz")
                    nc.scalar.activation(rz[:], pr["prz"][:], Act.Sigmoid)
                    pr["rz"] = rz
                for pr in prs:
                    m1 = m1p.tile([128, N], dt_h, tag="m1")
                    if first:
                        nc.vector.tensor_scalar(m1[:], pr["rz"][:, 0:N], bhn,
                                                None, Alu.mult)
                    else:
                        nc.vector.scalar_tensor_tensor(
                            m1[:], pr["pnh"][:], bhn, pr["rz"][:, 0:N],
                            Alu.add, Alu.mult)
                    pr["m1"] = m1
                for pr in prs:
                    mm(pr["pn"][:], i128, pr["m1"][:], False, True)
                for pr in prs:
                    ns = nsp.tile([128, N], dt_h, tag="ns")
                    nc.scalar.activation(ns[:], pr["pn"][:], Act.Tanh)
                    pr["ns"] = ns
                # ---- h' = h + w*(n - h) ----
                for pr in prs:
                    if last:
                        hn = hfp.tile([128, N], dt_h, tag="hf")
                    else:
                        hn = hp.tile([128, N], dt_h, tag="h")
                    if first:
                        nc.gpsimd.tensor_tensor(hn[:], pr["ns"][:],
                                                pr["rz"][:, N:2 * N], Alu.mult)
                    else:
                        u = upl.tile([128, N], dt_h, tag="u")
                        nc.gpsimd.tensor_tensor(u[:], pr["ns"][:], pr["h"][:],
                                                Alu.subtract)
                        v = vpl.tile([128, N], dt_h, tag="v")
                        nc.gpsimd.tensor_tensor(v[:], pr["rz"][:, N:2 * N],
                                                u[:], Alu.mult)
                        nc.vector.tensor_tensor(hn[:], pr["h"][:], v[:], Alu.add)
                    pr["h"] = hn
            # ---- FC head + one contiguous out-DMA per superblock ----
            for pr in prs:
                h = pr["h"]
                j = pr["j"]
                for g in range(2):
                    p_l = pnp.tile([2, N], f32, tag="pn")
                    mm(p_l[:], wfc[64 * g:64 * (g + 1), :],
                       h[64 * g:64 * (g + 1), :], True, True)
                    stg = st[0:2, j * PAIR + g * N:j * PAIR + (g + 1) * N]
                    nc.vector.tensor_scalar(stg, p_l[:], bfc, None, Alu.add)
            base = blk * IL * PAIR
            nc.sync.dma_start(out_d[:, base:base + IL * PAIR], st[:])

    nc.compile()
    return nc


def _host_constants(W_ih, W_hh, b_ih, b_hh, W_fc, b_fc, dt_h_np):
    f32 = np.float32
    cb = np.zeros((128, 1024), f32)
    cf = np.zeros((128, 8), f32)
    w_in = W_ih[:, 0].astype(f32)
    sgn = [1.0, -1.0, 1.0]
    for g in range(3):
        W = W_hh[64 * g:64 * (g + 1)].astype(f32) * sgn[g]   # [64, 64]
        cb[0:64, 128 * g:128 * g + 64] = W.T
        cb[64:128, 128 * g + 64:128 * g + 128] = W.T
        wg = w_in[64 * g:64 * (g + 1)] * sgn[g]
        col = 640 + 128 * g
        cb[0, col:col + 64] = wg
        cb[1, col + 64:col + 128] = wg
        if g < 2:
            bias = (b_ih[64 * g:64 * (g + 1)] + b_hh[64 * g:64 * (g + 1)]) * sgn[g]
        else:
            bias = b_ih[128:192]
        cb[2, col:col + 128] = np.concatenate([bias, bias])
    cb[:, 384:512] = np.eye(128, dtype=f32)
    cb[0:64, 512:514] = W_fc.T
    cb[64:128, 512:514] = W_fc.T
    cf[:, 0] = np.concatenate([b_hh[128:192]] * 2)
    cf[0:2, 1] = b_fc
    return {"cb": cb.astype(dt_h_np), "cf": cf}


def kernel(x, W_ih, W_hh, b_ih, b_hh, W_fc, b_fc, _trace=False, _trace_kwargs=None):
    import ml_dtypes

    from concourse.bass_utils import run_bass_kernel_spmd

    dt_h_name = "bfloat16"
    if dt_h_name not in _cache:
        _cache[dt_h_name] = _build(dt_h_name)
    nc = _cache[dt_h_name]

    consts = _host_constants(W_ih, W_hh, b_ih, b_hh, W_fc, b_fc,
                             ml_dtypes.bfloat16)
    x = np.ascontiguousarray(np.asarray(x, np.float32))
    in_maps = []
    for c in range(NCORES):
        m = {"x": x[c * BC:(c + 1) * BC]}
        m.update(consts)
        in_maps.append(m)
    kw = {}
    if _trace:
        kw["trace"] = True
        if _trace_kwargs:
            kw.update(_trace_kwargs)
    res = run_bass_kernel_spmd(nc, in_maps, list(range(NCORES)), **kw)
    out = np.concatenate(
        [np.ascontiguousarray(res.results[c]["out"].T) for c in range(NCORES)],
        axis=0)
    if _trace:
        return out, res
    return out


if __name__ == "__main__":
    rng = np.random.default_rng(0)
    s = 1.0 / np.sqrt(H)
    inputs = {
        "x": rng.standard_normal((B, T), dtype=np.float32),
        "W_ih": rng.uniform(-s, s, (3 * H, 1)).astype(np.float32),
        "W_hh": rng.uniform(-s, s, (3 * H, H)).astype(np.float32),
        "b_ih": rng.uniform(-s, s, (3 * H,)).astype(np.float32),
        "b_hh": rng.uniform(-s, s, (3 * H,)).astype(np.float32),
        "W_fc": rng.uniform(-s, s, (2, H)).astype(np.float32),
        "b_fc": rng.uniform(-s, s, (2,)).astype(np.float32),
    }
    out = kernel(**inputs)
    print(out.shape, out.dtype, out[:4])


# revision 6
# speedup vs baseline: 2.9083x; 1.5202x over previous
"""Trainium2 Bass kernel for nn_LungCancerGRU (GRU H=64, T=15, B=262144 -> logits [B,2]).

Data parallel over 8 NeuronCores (batch sharded, 32768 rows/core).

Per-core layout: gate units on SBUF partitions, batch on the free dimension.
Batch runs in pair-tiles of 1024 rows = two groups (A, B) of N=512; group A
occupies partitions 0..63, group B 64..127 of every [128, 512] tile.  All
matmul operands are bf16 with CONTIGUOUS rhs streams (fp32 and strided-rhs
matmuls run at a fraction of PE rate - avoid both).

x is transposed on the HOST to [T, BC] so the per-pair x tile loads t-major:
xt [3, 15*512] with xt[g, t*512+n] = x[base+g*512+n, t]; row 2 is a
persistent 1.0 bias lane.  Step t's x-part rhs = xt[:, t*512:(t+1)*512] is
contiguous at base partition 0 (matmul rhs must start at partition 0/32/64).

Per timestep t (per pair-tile), z sign-flipped host-side so one sigmoid
yields both r and w = 1-z:
  psum_rz[:, :512] = BD(W_hr^T) @ h + x3_r @ [x^A; x^B; 1]    (r preact+bias)
  psum_rz[:, 512:] = BD(-W_hz^T) @ h + x3_z @ ...             (-z preact-bias)
  psum_nh          = BD(W_hn^T) @ h                           (h-part of n)
  psum_n           = x3_n @ [x^A; x^B; 1]                     (x w_n + b_ih_n)
  rz  = sigmoid(psum_rz)     ONE ACT op [128,1024] -> [r | w]
  m1  = (psum_nh + b_hhn) * r     fused scalar_tensor_tensor on DVE
  psum_n += I128 @ m1             identity-matmul accumulate (PE)
  n   = tanh(psum_n)              ACT
  u = n - h (GpSimd); v = w*u (DVE); h' = h + v (DVE)

IL=4 pair-tiles run in lockstep so the ~7us per-step dependency chain
pipelines across pairs (PSUM slots recycle mid-chain).  The FC head + output
DMA of superblock b are emitted after step 1 of superblock b+1 so the PE
never stalls on the end-of-superblock chain drain.

Output is stored transposed in DRAM as [2, BC] (contiguous row DMAs) and
transposed back on the host.
"""

import sys

import numpy as np

sys.path.insert(0, "/opt/trn_rl_repo")

B, T, H = 262144, 15, 64
NCORES = 8
BC = B // NCORES          # 32768 rows per core
N = 512                   # batch columns per group
PAIR = 2 * N              # 1024 rows per pair-tile
NPAIR = BC // PAIR        # 32 pair-tiles per core
IL = 4                    # pair-tiles processed in lockstep
NSB = NPAIR // IL         # superblocks
XW = T * N                # xt tile free width (7680)
XSLOTS = 6                # persistent xt buffers

_cache = {}


def _build(dt_h_name):
    from contextlib import ExitStack

    import concourse.bacc as bacc
    import concourse.mybir as mybir
    from concourse.tile import TileContext

    f32 = mybir.dt.float32
    dt_h = getattr(mybir.dt, dt_h_name)
    Act = mybir.ActivationFunctionType
    Alu = mybir.AluOpType

    nc = bacc.Bacc(None)

    x_in = nc.dram_tensor("xT", [T, BC], f32, kind="ExternalInput")
    out_d = nc.dram_tensor("out", [2, BC], f32, kind="ExternalOutput")
    cb_in = nc.dram_tensor("cb", [128, 1024], dt_h, kind="ExternalInput")
    cf_in = nc.dram_tensor("cf", [128, 8], f32, kind="ExternalInput")

    with TileContext(nc) as tc, ExitStack() as es:
        # ---- constants: one DMA per dtype ----
        cpool = es.enter_context(tc.tile_pool(name="const", bufs=1))
        cb = cpool.tile([128, 1024], dt_h)
        nc.sync.dma_start(cb[:], cb_in[:])
        cf = cpool.tile([128, 8], f32)
        nc.sync.dma_start(cf[:], cf_in[:])

        bd = [cb[:, 128 * g:128 * (g + 1)] for g in range(3)]  # r, -z, n
        i128 = cb[:, 384:512]
        wfc = cb[:, 512:514]
        x3 = [cb[0:3, 640 + 128 * g:768 + 128 * g] for g in range(3)]
        bhn = cf[:, 0:1]
        bfc = cf[0:2, 1:2]

        # ---- pools ----
        xt_pool = es.enter_context(tc.tile_pool(name="xt", bufs=XSLOTS))
        hp = es.enter_context(tc.tile_pool(name="h", bufs=2 * IL))
        hfp = es.enter_context(tc.tile_pool(name="hf", bufs=IL))
        rzp = es.enter_context(tc.tile_pool(name="rz", bufs=2 * IL))
        m1p = es.enter_context(tc.tile_pool(name="m1", bufs=2 * IL))
        nsp = es.enter_context(tc.tile_pool(name="ns", bufs=2 * IL))
        upl = es.enter_context(tc.tile_pool(name="u", bufs=2 * IL))
        vpl = es.enter_context(tc.tile_pool(name="v", bufs=2 * IL))
        stp = es.enter_context(tc.tile_pool(name="st", bufs=2))
        przp = es.enter_context(tc.tile_pool(name="prz", bufs=2, space="PSUM"))
        pnhp = es.enter_context(tc.tile_pool(name="pnh", bufs=2, space="PSUM"))
        pnp = es.enter_context(tc.tile_pool(name="pn", bufs=2, space="PSUM"))

        def mm(out, lhsT, rhs, start, stop):
            nc.tensor.matmul(out, lhsT, rhs, start=start, stop=stop,
                             skip_group_check=True)

        # ---- persistent xt tiles; row 2 = 1.0 (bias lane), set once ----
        xts = []
        for i in range(XSLOTS):
            xt = xt_pool.tile([3, XW], dt_h, tag="xt", name="xt")
            # rows 0:2 are overwritten by every x DMA; row 2 stays 1.0
            nc.gpsimd.memset(xt[0:3, :], 1.0)
            xts.append(xt)

        # ---- engine warm-ups: fold the const-DMA sems into each engine's
        # clock once so steady-state instructions never re-wait on them.
        pwarm = pnp.tile([2, 2], f32, tag="pn")
        mm(pwarm[:], cb[0:2, 0:2], cb[0:2, 0:2], True, True)
        wt = cpool.tile([2, 8], f32)
        nc.vector.tensor_copy(wt[0:1, 0:1], cf[0:1, 0:1])
        nc.vector.tensor_copy(wt[0:1, 1:2], cb[0:1, 0:1])
        nc.scalar.copy(wt[0:1, 2:3], cf[0:1, 0:1])
        nc.scalar.copy(wt[0:1, 3:4], cb[0:1, 0:1])
        nc.gpsimd.tensor_copy(wt[0:1, 4:5], cb[0:1, 0:1])
        nc.gpsimd.tensor_copy(wt[0:1, 5:6], cf[0:1, 0:1])

        def emit_fc(prs):
            """FC head + one contiguous out-DMA for a finished superblock."""
            st = stp.tile([2, IL * PAIR], f32, tag="st", name="st")
            for pr in prs:
                h = pr["h"]
                j = pr["j"]
                for g in range(2):
                    p_l = pnp.tile([2, N], f32, tag="pn", name="pl")
                    mm(p_l[:], wfc[64 * g:64 * (g + 1), :],
                       h[64 * g:64 * (g + 1), :], True, True)
                    stg = st[0:2, j * PAIR + g * N:j * PAIR + (g + 1) * N]
                    nc.vector.tensor_scalar(stg, p_l[:], bfc, None, Alu.add)
            base = prs[0]["blk"] * IL * PAIR
            nc.sync.dma_start(out_d[:, base:base + IL * PAIR], st[:])

        prev_prs = None
        for blk in range(NSB):
            prs = []
            for j in range(IL):
                p = blk * IL + j
                xt = xts[p % XSLOTS]
                # one cast DMA (f32 -> bf16), 2KB-contiguous runs
                src = x_in[:, p * PAIR:(p + 1) * PAIR]
                nc.gpsimd.dma_start(
                    xt[0:2, :].rearrange("g (t n) -> g t n", t=T),
                    src.rearrange("t (g n) -> g t n", g=2))
                prs.append({"xt": xt, "h": None, "j": j, "blk": blk})
            for t in range(T):
                first = t == 0
                last = t == T - 1
                xc = slice(t * N, (t + 1) * N)
                for pr in prs:
                    pr["prz"] = przp.tile([128, 2 * N], f32, tag="prz", name="prz")
                    pr["pn"] = pnp.tile([128, N], f32, tag="pn", name="pn")
                # ---- PE streams, weight-grouped across the IL pairs ----
                if not first:
                    for pr in prs:
                        mm(pr["prz"][:, 0:N], bd[0], pr["h"][:], True, False)
                    for pr in prs:
                        mm(pr["prz"][:, N:2 * N], bd[1], pr["h"][:], True, False)
                for pr in prs:
                    mm(pr["prz"][:, 0:N], x3[0], pr["xt"][0:3, xc], first, True)
                for pr in prs:
                    mm(pr["prz"][:, N:2 * N], x3[1], pr["xt"][0:3, xc], first, True)
                # ---- sigmoid -> [r | w] ----
                for pr in prs:
                    rz = rzp.tile([128, 2 * N], dt_h, tag="rz", name="rz")
                    nc.scalar.activation(rz[:], pr["prz"][:], Act.Sigmoid)
                    pr["rz"] = rz
                if not first:
                    for pr in prs:
                        pr["pnh"] = pnhp.tile([128, N], f32, tag="pnh", name="pnh")
                        mm(pr["pnh"][:], bd[2], pr["h"][:], True, True)
                for pr in prs:
                    mm(pr["pn"][:], x3[2], pr["xt"][0:3, xc], True, False)
                # ---- m1 = (nh + b_hhn) * r ----
                for pr in prs:
                    m1 = m1p.tile([128, N], dt_h, tag="m1", name="m1")
                    if first:
                        nc.vector.tensor_scalar(m1[:], pr["rz"][:, 0:N], bhn,
                                                None, Alu.mult)
                    else:
                        nc.vector.scalar_tensor_tensor(
                            m1[:], pr["pnh"][:], bhn, pr["rz"][:, 0:N],
                            Alu.add, Alu.mult)
                    pr["m1"] = m1
                for pr in prs:
                    mm(pr["pn"][:], i128, pr["m1"][:], False, True)
                for pr in prs:
                    ns = nsp.tile([128, N], dt_h, tag="ns", name="ns")
                    nc.scalar.activation(ns[:], pr["pn"][:], Act.Tanh)
                    pr["ns"] = ns
                # ---- h' = h + w*(n - h) ----
                for pr in prs:
                    if last:
                        hn = hfp.tile([128, N], dt_h, tag="hf", name="hf")
                    else:
                        hn = hp.tile([128, N], dt_h, tag="h", name="h")
                    if first:
                        nc.gpsimd.tensor_tensor(hn[:], pr["ns"][:],
                                                pr["rz"][:, N:2 * N], Alu.mult)
                    else:
                        u = upl.tile([128, N], dt_h, tag="u", name="u")
                        nc.gpsimd.tensor_tensor(u[:], pr["ns"][:], pr["h"][:],
                                                Alu.subtract)
                        v = vpl.tile([128, N], dt_h, tag="v", name="v")
                        nc.vector.tensor_tensor(v[:], pr["rz"][:, N:2 * N],
                                                u[:], Alu.mult)
                        nc.vector.tensor_tensor(hn[:], pr["h"][:], v[:], Alu.add)
                    pr["h"] = hn
                # FC of the previous superblock rides in here so the PE
                # keeps streaming while that superblock's chain drains.
                if t == 1 and prev_prs is not None:
                    emit_fc(prev_prs)
                    prev_prs = None
            prev_prs = prs
        emit_fc(prev_prs)

    nc.compile()
    return nc


def _host_constants(W_ih, W_hh, b_ih, b_hh, W_fc, b_fc, dt_h_np):
    f32 = np.float32
    cb = np.zeros((128, 1024), f32)
    cf = np.zeros((128, 8), f32)
    w_in = W_ih[:, 0].astype(f32)
    sgn = [1.0, -1.0, 1.0]
    for g in range(3):
        W = W_hh[64 * g:64 * (g + 1)].astype(f32) * sgn[g]   # [64, 64]
        cb[0:64, 128 * g:128 * g + 64] = W.T
        cb[64:128, 128 * g + 64:128 * g + 128] = W.T
        wg = w_in[64 * g:64 * (g + 1)] * sgn[g]
        col = 640 + 128 * g
        cb[0, col:col + 64] = wg
        cb[1, col + 64:col + 128] = wg
        if g < 2:
            bias = (b_ih[64 * g:64 * (g + 1)] + b_hh[64 * g:64 * (g + 1)]) * sgn[g]
        else:
            bias = b_ih[128:192]
        cb[2, col:col + 128] = np.concatenate([bias, bias])
    cb[:, 384:512] = np.eye(128, dtype=f32)
    cb[0:64, 512:514] = W_fc.T
    cb[64:128, 512:514] = W_fc.T
    cf[:, 0] = np.concatenate([b_hh[128:192]] * 2)
    cf[0:2, 1] = b_fc
    return {"cb": cb.astype(dt_h_np), "cf": cf}


def kernel(x, W_ih, W_hh, b_ih, b_hh, W_fc, b_fc, _trace=False, _trace_kwargs=None):
    import ml_dtypes

    from concourse.bass_utils import run_bass_kernel_spmd

    dt_h_name = "bfloat16"
    if dt_h_name not in _cache:
        _cache[dt_h_name] = _build(dt_h_name)
    nc = _cache[dt_h_name]

    consts = _host_constants(W_ih, W_hh, b_ih, b_hh, W_fc, b_fc,
                             ml_dtypes.bfloat16)
    xT = np.ascontiguousarray(np.asarray(x, np.float32).T)   # [T, B]
    in_maps = []
    for c in range(NCORES):
        m = {"xT": np.ascontiguousarray(xT[:, c * BC:(c + 1) * BC])}
        m.update(consts)
        in_maps.append(m)
    kw = {}
    if _trace:
        kw["trace"] = True
        if _trace_kwargs:
            kw.update(_trace_kwargs)
    res = run_bass_kernel_spmd(nc, in_maps, list(range(NCORES)), **kw)
    out = np.concatenate(
        [np.ascontiguousarray(res.results[c]["out"].T) for c in range(NCORES)],
        axis=0)
    if _trace:
        return out, res
    return out


if __name__ == "__main__":
    rng = np.random.default_rng(0)
    s = 1.0 / np.sqrt(H)
    inputs = {
        "x": rng.standard_normal((B, T), dtype=np.float32),
        "W_ih": rng.uniform(-s, s, (3 * H, 1)).astype(np.float32),
        "W_hh": rng.uniform(-s, s, (3 * H, H)).astype(np.float32),
        "b_ih": rng.uniform(-s, s, (3 * H,)).astype(np.float32),
        "b_hh": rng.uniform(-s, s, (3 * H,)).astype(np.float32),
        "W_fc": rng.uniform(-s, s, (2, H)).astype(np.float32),
        "b_fc": rng.uniform(-s, s, (2,)).astype(np.float32),
    }
    out = kernel(**inputs)
    print(out.shape, out.dtype, out[:4])


# revision 7
# speedup vs baseline: 3.5084x; 1.2063x over previous
"""Trainium2 Bass kernel for nn_LungCancerGRU (GRU H=64, T=15, B=262144 -> logits [B,2]).

Data parallel over 8 NeuronCores (batch sharded, 32768 rows/core).

Per-core layout: gate units on SBUF partitions, batch on the free dimension.
Batch runs in pair-tiles of 1024 rows = two groups (A, B) of N=512; group A
occupies partitions 0..63, group B 64..127 of every [128, 512] tile.  All
matmul operands are bf16 with CONTIGUOUS rhs streams (fp32 and strided-rhs
matmuls run at a fraction of PE rate - avoid both).

x is transposed on the HOST to [T, BC] so the per-pair x tile loads t-major:
xt [3, 15*512] with xt[g, t*512+n] = x[base+g*512+n, t]; row 2 is a
persistent 1.0 bias lane.  Step t's x-part rhs = xt[:, t*512:(t+1)*512] is
contiguous at base partition 0 (matmul rhs must start at partition 0/32/64).

Per timestep t (per pair-tile), z sign-flipped host-side so one sigmoid
yields both r and w = 1-z:
  psum_rz[:, :512] = BD(W_hr^T) @ h + x3_r @ [x^A; x^B; 1]    (r preact+bias)
  psum_rz[:, 512:] = BD(-W_hz^T) @ h + x3_z @ ...             (-z preact-bias)
  psum_nh          = BD(W_hn^T) @ h                           (h-part of n)
  psum_n           = x3_n @ [x^A; x^B; 1]                     (x w_n + b_ih_n)
  rz  = sigmoid(psum_rz)     ONE ACT op [128,1024] -> [r | w]
  m1  = (psum_nh + b_hhn) * r     fused scalar_tensor_tensor on DVE
  psum_n += I128 @ m1             identity-matmul accumulate (PE)
  n   = tanh(psum_n)              ACT
  u = n - h (GpSimd); v = w*u (DVE); h' = h + v (DVE)

IL=4 pair-tiles run in lockstep so the ~7us per-step dependency chain
pipelines across pairs (PSUM slots recycle mid-chain).  The FC head + output
DMA of superblock b are emitted after step 1 of superblock b+1 so the PE
never stalls on the end-of-superblock chain drain.

Output is stored transposed in DRAM as [2, BC] (contiguous row DMAs) and
transposed back on the host.
"""

import sys

import numpy as np

sys.path.insert(0, "/opt/trn_rl_repo")

B, T, H = 262144, 15, 64
NCORES = 8
BC = B // NCORES          # 32768 rows per core
N = 512                   # batch columns per group
PAIR = 2 * N              # 1024 rows per pair-tile
NPAIR = BC // PAIR        # 32 pair-tiles per core
IL = 4                    # pair-tiles processed in lockstep
NSB = NPAIR // IL         # superblocks
XW = T * N                # xt tile free width (7680)
XSLOTS = 6                # persistent xt buffers

_cache = {}


def _build(dt_h_name):
    from contextlib import ExitStack

    import concourse.bacc as bacc
    import concourse.mybir as mybir
    from concourse.tile import TileContext

    f32 = mybir.dt.float32
    dt_h = getattr(mybir.dt, dt_h_name)
    Act = mybir.ActivationFunctionType
    Alu = mybir.AluOpType

    nc = bacc.Bacc(None)

    x_in = nc.dram_tensor("xT", [T, BC], f32, kind="ExternalInput")
    out_d = nc.dram_tensor("out", [2, BC], f32, kind="ExternalOutput")
    cb_in = nc.dram_tensor("cb", [128, 1024], dt_h, kind="ExternalInput")
    cf_in = nc.dram_tensor("cf", [128, 8], f32, kind="ExternalInput")

    with TileContext(nc) as tc, ExitStack() as es:
        # ---- constants: one DMA per dtype ----
        cpool = es.enter_context(tc.tile_pool(name="const", bufs=1))
        cb = cpool.tile([128, 1024], dt_h)
        nc.sync.dma_start(cb[:], cb_in[:])
        cf = cpool.tile([128, 8], f32)
        nc.sync.dma_start(cf[:], cf_in[:])

        bd = [cb[:, 128 * g:128 * (g + 1)] for g in range(3)]  # r, -z, n
        i128 = cb[:, 384:512]
        wfc = cb[:, 512:514]
        x3 = [cb[32 * g:32 * g + 3, 640:768] for g in range(3)]
        bhn = cf[:, 0:1]
        bfc = cf[0:2, 1:2]

        # ---- pools ----
        xt_pool = es.enter_context(tc.tile_pool(name="xt", bufs=XSLOTS))
        hp = es.enter_context(tc.tile_pool(name="h", bufs=2 * IL))
        hfp = es.enter_context(tc.tile_pool(name="hf", bufs=IL))
        rzp = es.enter_context(tc.tile_pool(name="rz", bufs=2 * IL))
        m1p = es.enter_context(tc.tile_pool(name="m1", bufs=2 * IL))
        nsp = es.enter_context(tc.tile_pool(name="ns", bufs=2 * IL))
        upl = es.enter_context(tc.tile_pool(name="u", bufs=2 * IL))
        vpl = es.enter_context(tc.tile_pool(name="v", bufs=2 * IL))
        stp = es.enter_context(tc.tile_pool(name="st", bufs=2))
        przp = es.enter_context(tc.tile_pool(name="prz", bufs=2, space="PSUM"))
        pnhp = es.enter_context(tc.tile_pool(name="pnh", bufs=2, space="PSUM"))
        pnp = es.enter_context(tc.tile_pool(name="pn", bufs=2, space="PSUM"))

        def mm(out, lhsT, rhs, start, stop):
            nc.tensor.matmul(out, lhsT, rhs, start=start, stop=stop,
                             skip_group_check=True)

        # ---- persistent xt tiles; row 2 = 1.0 (bias lane), set once ----
        xts = []
        for i in range(XSLOTS):
            xt = xt_pool.tile([67, XW], dt_h, tag="xt", name="xt")
            # x lives at rows {0:2, 32:34, 64:66} (one copy per PE row-group
            # so the three gate x-matmuls run concurrently row-tiled); the
            # all-ones bias lane is row 2/34/66.  DMAs overwrite the x rows;
            # everything else stays 1.0 from this one-time fill.
            nc.gpsimd.memset(xt[0:67, :], 1.0)
            xts.append(xt)

        # ---- engine warm-ups: fold the const-DMA sems into each engine's
        # clock once so steady-state instructions never re-wait on them.
        pwarm = pnp.tile([2, 2], f32, tag="pn")
        mm(pwarm[:], cb[0:2, 0:2], cb[0:2, 0:2], True, True)
        wt = cpool.tile([2, 8], f32)
        nc.vector.tensor_copy(wt[0:1, 0:1], cf[0:1, 0:1])
        nc.vector.tensor_copy(wt[0:1, 1:2], cb[0:1, 0:1])
        nc.scalar.copy(wt[0:1, 2:3], cf[0:1, 0:1])
        nc.scalar.copy(wt[0:1, 3:4], cb[0:1, 0:1])
        nc.gpsimd.tensor_copy(wt[0:1, 4:5], cb[0:1, 0:1])
        nc.gpsimd.tensor_copy(wt[0:1, 5:6], cf[0:1, 0:1])

        def emit_fc(prs):
            """FC head + one contiguous out-DMA for a finished superblock."""
            st = stp.tile([2, IL * PAIR], f32, tag="st", name="st")
            for pr in prs:
                h = pr["h"]
                j = pr["j"]
                for g in range(2):
                    p_l = pnp.tile([2, N], f32, tag="pn", name="pl")
                    mm(p_l[:], wfc[64 * g:64 * (g + 1), :],
                       h[64 * g:64 * (g + 1), :], True, True)
                    stg = st[0:2, j * PAIR + g * N:j * PAIR + (g + 1) * N]
                    nc.vector.tensor_scalar(stg, p_l[:], bfc, None, Alu.add)
            base = prs[0]["blk"] * IL * PAIR
            nc.sync.dma_start(out_d[:, base:base + IL * PAIR], st[:])

        prev_prs = None
        for blk in range(NSB):
            prs = []
            for j in range(IL):
                p = blk * IL + j
                xt = xts[p % XSLOTS]
                # cast DMAs (f32 -> bf16), 2KB-contiguous runs; one copy per
                # PE row-group band
                src = x_in[:, p * PAIR:(p + 1) * PAIR]
                for band in range(3):
                    nc.gpsimd.dma_start(
                        xt[32 * band:32 * band + 2, :].rearrange(
                            "g (t n) -> g t n", t=T),
                        src.rearrange("t (g n) -> g t n", g=2))
                prs.append({"xt": xt, "h": None, "j": j, "blk": blk})
            for t in range(T):
                first = t == 0
                last = t == T - 1
                xc = slice(t * N, (t + 1) * N)
                for pr in prs:
                    pr["prz"] = przp.tile([128, 2 * N], f32, tag="prz", name="prz")
                    pr["pn"] = pnp.tile([128, N], f32, tag="pn", name="pn")
                # ---- PE: x-part trio first (no h dependency; the three
                # gate matmuls run concurrently in row-groups 0/1/2), then
                # the h-part matmuls accumulate on top ----
                for pr in prs:
                    mm(pr["prz"][:, 0:N], x3[0], pr["xt"][0:3, xc],
                       True, first)
                    mm(pr["prz"][:, N:2 * N], x3[1], pr["xt"][32:35, xc],
                       True, first)
                    mm(pr["pn"][:], x3[2], pr["xt"][64:67, xc], True, False)
                if not first:
                    for pr in prs:
                        mm(pr["prz"][:, 0:N], bd[0], pr["h"][:], False, True)
                    for pr in prs:
                        mm(pr["prz"][:, N:2 * N], bd[1], pr["h"][:], False, True)
                # ---- sigmoid -> [r | w] ----
                for pr in prs:
                    rz = rzp.tile([128, 2 * N], dt_h, tag="rz", name="rz")
                    nc.scalar.activation(rz[:], pr["prz"][:], Act.Sigmoid)
                    pr["rz"] = rz
                if not first:
                    for pr in prs:
                        pr["pnh"] = pnhp.tile([128, N], f32, tag="pnh", name="pnh")
                        mm(pr["pnh"][:], bd[2], pr["h"][:], True, True)
                # ---- m1 = (nh + b_hhn) * r ----
                for pr in prs:
                    m1 = m1p.tile([128, N], dt_h, tag="m1", name="m1")
                    if first:
                        nc.vector.tensor_scalar(m1[:], pr["rz"][:, 0:N], bhn,
                                                None, Alu.mult)
                    else:
                        nc.vector.scalar_tensor_tensor(
                            m1[:], pr["pnh"][:], bhn, pr["rz"][:, 0:N],
                            Alu.add, Alu.mult)
                    pr["m1"] = m1
                for pr in prs:
                    mm(pr["pn"][:], i128, pr["m1"][:], False, True)
                for pr in prs:
                    ns = nsp.tile([128, N], dt_h, tag="ns", name="ns")
                    nc.scalar.activation(ns[:], pr["pn"][:], Act.Tanh)
                    pr["ns"] = ns
                # ---- h' = h + w*(n - h) ----
                for pr in prs:
                    if last:
                        hn = hfp.tile([128, N], dt_h, tag="hf", name="hf")
                    else:
                        hn = hp.tile([128, N], dt_h, tag="h", name="h")
                    if first:
                        nc.gpsimd.tensor_tensor(hn[:], pr["ns"][:],
                                                pr["rz"][:, N:2 * N], Alu.mult)
                    else:
                        u = upl.tile([128, N], dt_h, tag="u", name="u")
                        nc.gpsimd.tensor_tensor(u[:], pr["ns"][:], pr["h"][:],
                                                Alu.subtract)
                        v = vpl.tile([128, N], dt_h, tag="v", name="v")
                        nc.vector.tensor_tensor(v[:], pr["rz"][:, N:2 * N],
                                                u[:], Alu.mult)
                        nc.vector.tensor_tensor(hn[:], pr["h"][:], v[:], Alu.add)
                    pr["h"] = hn
                # FC of the previous superblock rides in here so the PE
                # keeps streaming while that superblock's chain drains.
                if t == 1 and prev_prs is not None:
                    emit_fc(prev_prs)
                    prev_prs = None
            prev_prs = prs
        emit_fc(prev_prs)

    nc.compile()
    return nc


def _host_constants(W_ih, W_hh, b_ih, b_hh, W_fc, b_fc, dt_h_np):
    f32 = np.float32
    cb = np.zeros((128, 1024), f32)
    cf = np.zeros((128, 8), f32)
    w_in = W_ih[:, 0].astype(f32)
    sgn = [1.0, -1.0, 1.0]
    for g in range(3):
        W = W_hh[64 * g:64 * (g + 1)].astype(f32) * sgn[g]   # [64, 64]
        cb[0:64, 128 * g:128 * g + 64] = W.T
        cb[64:128, 128 * g + 64:128 * g + 128] = W.T
        wg = w_in[64 * g:64 * (g + 1)] * sgn[g]
        row = 32 * g
        cb[row, 640:640 + 64] = wg
        cb[row + 1, 640 + 64:640 + 128] = wg
        if g < 2:
            bias = (b_ih[64 * g:64 * (g + 1)] + b_hh[64 * g:64 * (g + 1)]) * sgn[g]
        else:
            bias = b_ih[128:192]
        cb[row + 2, 640:640 + 128] = np.concatenate([bias, bias])
    cb[:, 384:512] = np.eye(128, dtype=f32)
    cb[0:64, 512:514] = W_fc.T
    cb[64:128, 512:514] = W_fc.T
    cf[:, 0] = np.concatenate([b_hh[128:192]] * 2)
    cf[0:2, 1] = b_fc
    return {"cb": cb.astype(dt_h_np), "cf": cf}


def kernel(x, W_ih, W_hh, b_ih, b_hh, W_fc, b_fc, _trace=False, _trace_kwargs=None):
    import ml_dtypes

    from concourse.bass_utils import run_bass_kernel_spmd

    dt_h_name = "bfloat16"
    if dt_h_name not in _cache:
        _cache[dt_h_name] = _build(dt_h_name)
    nc = _cache[dt_h_name]

    consts = _host_constants(W_ih, W_hh, b_ih, b_hh, W_fc, b_fc,
                             ml_dtypes.bfloat16)
    xT = np.ascontiguousarray(np.asarray(x, np.float32).T)   # [T, B]
    in_maps = []
    for c in range(NCORES):
        m = {"xT": np.ascontiguousarray(xT[:, c * BC:(c + 1) * BC])}
        m.update(consts)
        in_maps.append(m)
    kw = {}
    if _trace:
        kw["trace"] = True
        if _trace_kwargs:
            kw.update(_trace_kwargs)
    res = run_bass_kernel_spmd(nc, in_maps, list(range(NCORES)), **kw)
    out = np.concatenate(
        [np.ascontiguousarray(res.results[c]["out"].T) for c in range(NCORES)],
        axis=0)
    if _trace:
        return out, res
    return out


if __name__ == "__main__":
    rng = np.random.default_rng(0)
    s = 1.0 / np.sqrt(H)
    inputs = {
        "x": rng.standard_normal((B, T), dtype=np.float32),
        "W_ih": rng.uniform(-s, s, (3 * H, 1)).astype(np.float32),
        "W_hh": rng.uniform(-s, s, (3 * H, H)).astype(np.float32),
        "b_ih": rng.uniform(-s, s, (3 * H,)).astype(np.float32),
        "b_hh": rng.uniform(-s, s, (3 * H,)).astype(np.float32),
        "W_fc": rng.uniform(-s, s, (2, H)).astype(np.float32),
        "b_fc": rng.uniform(-s, s, (2,)).astype(np.float32),
    }
    out = kernel(**inputs)
    print(out.shape, out.dtype, out[:4])
